# revision 1
# baseline (speedup 1.0000x reference)
"""ArcFace-style loss on 8 TRN2 NeuronCores — v5: 4-bit-packed W.

This environment's HBM->SBUF DMA sustains only ~22 GB/s per core (measured;
shared across both hwdge queues), so v4 (fp8 W, 9.8 MB/core) is DMA-bound at
~450 us. v5 halves the stream: W ships as 4-bit codes (2 per byte, 4.9 MB),
unpacked on the otherwise-idle DVE engine into exactly-representable
fp8 values c*0.25; the code offset (-7.5) and the quantizer scale fold into
the per-batch-row ACT bias and scale, so the algebra is exact given the
quantized operands:

  w_hat ~ (c - 7.5) * D0,  c in [0,15]   (4-bit quantizer, D0 = 0.325/sqrt(D))
  device: psum = sum_k (32*a_hat)_k * (0.25*c)_k
          Z_part = exp(ALPHA * psum + zb[row]),  zb = -ALPHA*1.875*32*sum_k a8
  => Z_part = exp(20 * cos_quantized)           (exactly)

Quantization error (fp8 a, 4-bit W) measured end-to-end: rel 2.3e-4 on the
loss, vs the 2e-2 gate.

Pipeline per core: packed W DMA chunks (4 windows) -> DVE unpacks both nibbles -> fp8 DoubleRow matmuls (stationary reused j-outer) ->
ACT exp+accum over [128, 2048] -> zacc -> host f64 epilogue (padding rows are
code 0 -> exp(zb), subtracted exactly; label corrections as usual).
"""

import numpy as np
import ml_dtypes

B = 1024
D = 768
C = 100000
NCORES = 8
SUB = D // 128            # 6 contraction subtiles
NW = 512                  # classes per PSUM bank
GRP = 4                   # windows per ACT op / psum tile
MARGIN = 0.4
SCALE = 20.0
EPS = 1e-07
SA = 32.0                              # fp8 pre-scale for a_hat
D0 = 0.325 / np.sqrt(D)                # 4-bit quantizer step for w_hat
ALPHA = SCALE * D0 / (SA * 0.25)       # ACT scale
ZB_COEF = -ALPHA * 1.875               # per-row bias coef (s_at already has SA)

CS = C // NCORES                      # 12500
CSP = ((CS + NW - 1) // NW) * NW      # 12800
NWIN = CSP // NW                      # 25

_CACHE: dict = {}


def _groups(nwin):
    gs, t = [], 0
    while t < nwin:
        g = min(GRP, nwin - t)
        gs.append((t, g))
        t += g
    return gs


def build_kernel(csp, reps=1):
    """reps>1: timing variant — full kernel body repeated inside one program."""
    import concourse.mybir as mybir
    import concourse.tile as tile
    from concourse import bacc

    dt = mybir.dt
    nwin = csp // NW
    nbt = B // 128
    groups = _groups(nwin)
    nsw = len(groups)

    nc = bacc.Bacc(None, target_bir_lowering=False)
    at_ext = nc.declare_dram_parameter("at", [128, SUB * B], dt.float8e4, isOutput=False)
    wp_ext = nc.declare_dram_parameter("wp", [128, nwin * (SUB // 2) * NW], dt.uint8, isOutput=False)
    zb_ext = nc.declare_dram_parameter("zb", [128, nbt], dt.float32, isOutput=False)
    out_ext = nc.declare_dram_parameter("out", [128, nsw * nbt], dt.float32, isOutput=True)

    with tile.TileContext(nc) as tc:
        with (
            tc.tile_pool(name="atp", bufs=2) as at_pool,
            tc.tile_pool(name="zp", bufs=2) as z_pool,
            tc.tile_pool(name="wload", bufs=3) as w_pool,
            tc.tile_pool(name="wup", bufs=2) as wu_pool,
            tc.tile_pool(name="scr", bufs=2) as sc_pool,
            tc.tile_pool(name="ps", bufs=2, space="PSUM") as ps_pool,
        ):
            for _ in range(reps):
                at = at_pool.tile([128, SUB, B], dt.float8e4, tag="at")
                nc.scalar.dma_start(out=at[:, :, :], in_=at_ext[:, :])
                zb = at_pool.tile([128, nbt], dt.float32, tag="zb")
                nc.scalar.dma_start(out=zb[:, :], in_=zb_ext[:, :])
                zacc = z_pool.tile([128, nsw * nbt], dt.float32, tag="zacc")

                for s, (t0, g) in enumerate(groups):
                    wp = w_pool.tile([128, GRP, SUB // 2, NW], dt.uint8, tag="wp")
                    nc.sync.dma_start(
                        out=wp[:, :g, :, :],
                        in_=wp_ext[:, t0 * (SUB // 2) * NW:(t0 + g) * (SUB // 2) * NW],
                    )
                    # unpack nibbles -> fp8 values c*0.25 (exact in e4m3).
                    # bitop and arith can't fuse in one tensor_scalar (walrus
                    # birverifier), and DVE writes must be contiguous (4D
                    # strided outs crash the exec unit), so the pair dim is
                    # FIRST in wt: each nibble plane is a contiguous prefix.
                    wt = wu_pool.tile([128, 2, GRP, SUB // 2, NW], dt.float8e4, tag="wt")
                    tlo = wu_pool.tile([128, GRP, SUB // 2, NW], dt.uint8, tag="tlo")
                    thi = wu_pool.tile([128, GRP, SUB // 2, NW], dt.uint8, tag="thi")
                    nc.vector.tensor_scalar(
                        tlo[:, :g, :, :], wp[:, :g, :, :],
                        15, None, mybir.AluOpType.bitwise_and,
                    )
                    nc.vector.tensor_scalar(
                        wt[:, 0, :g, :, :], tlo[:, :g, :, :],
                        0.25, None, mybir.AluOpType.mult,
                    )
                    nc.vector.tensor_scalar(
                        thi[:, :g, :, :], wp[:, :g, :, :],
                        4, None, mybir.AluOpType.logical_shift_right,
                    )
                    nc.vector.tensor_scalar(
                        wt[:, 1, :g, :, :], thi[:, :g, :, :],
                        0.25, None, mybir.AluOpType.mult,
                    )
                    for bt in range(nbt):
                        ps = ps_pool.tile([128, GRP * NW], dt.float32, tag="ps")
                        for j in range(SUB // 2):
                            for q in range(g):
                                nc.tensor.matmul(
                                    ps[:, q * NW:(q + 1) * NW],
                                    at[:, 2 * j:2 * j + 2, bt * 128:(bt + 1) * 128],
                                    wt[:, :, q, j, :],
                                    start=(j == 0), stop=(j == SUB // 2 - 1),
                                    perf_mode=mybir.MatmulPerfMode.DoubleRow,
                                )
                        sc = sc_pool.tile([128, GRP * NW], dt.bfloat16, tag="sc")
                        nc.scalar.activation(
                            sc[:, :g * NW], ps[:, :g * NW],
                            mybir.ActivationFunctionType.Exp,
                            scale=ALPHA,
                            bias=zb[:, bt:bt + 1],
                            accum_out=zacc[:, s * nbt + bt:s * nbt + bt + 1],
                        )

                nc.sync.dma_start(out=out_ext[:, :], in_=zacc[:])

    return nc


def _get_graph(csp, reps=1):
    key = (csp, reps)
    if key not in _CACHE:
        nc = build_kernel(csp, reps)
        nc.finalize()
        _CACHE[key] = nc
    return _CACHE[key]


def _prep_at(embeddings):
    emb = np.asarray(embeddings, dtype=np.float32)
    an = emb / np.linalg.norm(emb, axis=1, keepdims=True)
    at8 = (SA * an).astype(ml_dtypes.float8_e4m3)       # [B, D]
    atT = np.ascontiguousarray(at8.T)                   # [D, B]
    at_r = atT.reshape(SUB, 128, B).transpose(1, 0, 2).reshape(128, SUB * B)
    # per-row bias zb[b] = ZB_COEF * sum_k a8[b, k] (f64 for exactness)
    s_at = at8.astype(np.float64).sum(axis=1)
    zb = (ZB_COEF * s_at).astype(np.float32)            # [B]
    nbt = B // 128
    zb_r = np.ascontiguousarray(zb.reshape(nbt, 128).T) # [128, nbt]
    return np.ascontiguousarray(at_r), zb_r, zb, an


def _prep_w(W, csp):
    """4-bit codes, packed 2/byte: lo nibble = even pair element (i=0)."""
    Wf = np.asarray(W, dtype=np.float32)
    n = np.linalg.norm(Wf, axis=1, keepdims=True)
    Wn = Wf / n
    codes = np.clip(np.round(Wn / D0 + 7.5), 0, 15).astype(np.uint8)  # [C, D]
    nwin = csp // NW
    shards = []
    for c in range(NCORES):
        sh = np.zeros((csp, D), dtype=np.uint8)         # pad rows -> code 0
        sh[:CS] = codes[c * CS:(c + 1) * CS]
        cT = np.ascontiguousarray(sh.T)                 # [D, csp]
        c5 = cT.reshape(SUB // 2, 2, 128, nwin, NW)     # [jj, i, p, t, n]
        packed = (c5[:, 0] | (c5[:, 1] << 4))           # [jj, p, t, n]
        pr = packed.transpose(1, 2, 0, 3).reshape(128, nwin * (SUB // 2) * NW)
        shards.append(np.ascontiguousarray(pr))
    return shards


def make_in_maps(embeddings, W, csp):
    at_r, zb_r, zb, an = _prep_at(embeddings)
    shards = _prep_w(W, csp)
    in_maps = [{"at": at_r, "wp": shards[c], "zb": zb_r} for c in range(NCORES)]
    return in_maps, (an, zb)


def finalize(results, aux, W, labels, csp):
    an, zb = aux
    Wf = np.asarray(W, dtype=np.float32)
    labels = np.asarray(labels).astype(np.int64)
    nwin = csp // NW
    nsw = len(_groups(nwin))
    nbt = B // 128
    Z = np.zeros(B, dtype=np.float64)
    for r in results:
        o = r["out"].astype(np.float64).reshape(128, nsw, nbt).sum(axis=1)
        Z += o.T.reshape(B)
    # padding rows are all-zero codes -> each contributes exp(0 + zb[b])
    Z -= float(NCORES * (csp - CS)) * np.exp(zb.astype(np.float64))

    wl = Wf[labels]
    wln = wl / np.linalg.norm(wl, axis=1, keepdims=True)
    cos_l = np.sum(an.astype(np.float64) * wln.astype(np.float64), axis=1)
    cos_l = np.clip(cos_l, -1.0 + EPS, 1.0 - EPS)
    t = np.cos(np.arccos(cos_l) + MARGIN) * SCALE
    Z = Z - np.exp(SCALE * cos_l) + np.exp(t)
    loss = np.mean(np.log(Z) - t)
    return np.asarray(loss, dtype=np.float32)


def kernel(embeddings, labels, W):
    from concourse.bass_utils import run_bass_kernel_spmd

    nc = _get_graph(CSP)
    in_maps, aux = make_in_maps(embeddings, W, CSP)
    res = run_bass_kernel_spmd(nc, in_maps, core_ids=list(range(NCORES)))
    return finalize(res.results, aux, W, labels, CSP)



# revision 2
# speedup vs baseline: 2.6703x; 2.6703x over previous
"""ArcFace-style loss on 8 TRN2 NeuronCores — v6: fp8 W, no unpack.

v5 shipped W as 4-bit codes and unpacked on DVE because RPC-polluted
measurements suggested ~22 GB/s/core DMA. Careful reps/batch scaling shows
the steady-state DMA cost here is per-partition-line (~330 GB/s effective
for [128, X] transfers): a dma_only ablation of the v5 body runs at ~35us
while the full kernel runs ~230us — compute-bound, with the 4 DVE unpack
passes (~160us serial on DVE) the largest single contributor.

v6 ships W as fp8 e4m3 directly (2x the DMA bytes of v5, still cheap) and
deletes the unpack entirely:

  a8 = fp8(SA * a_normalized)   [B, D]    SA = 32
  w8 = fp8(SW * w_normalized)   [C, D]    SW = 16
  device: psum = sum_k a8_k w8_k; Z_part = exp(ALPHA * psum), ALPHA = 20/(SA*SW)
  accumulated per 128-row tile into zacc; host f64 epilogue subtracts the
  padding contribution (w8 = 0 -> exp(0) = 1 each) and applies the exact
  label-class margin corrections.

Pipeline per core: fp8 W DMA chunks (4 windows) -> fp8 DoubleRow matmuls
(a stationary, reused across windows) -> ACT exp+accum over [128, 2048].
"""

import numpy as np
import ml_dtypes

B = 1024
D = 768
C = 100000
NCORES = 8
SUB = D // 128            # 6 contraction subtiles
NW = 512                  # classes per PSUM bank
GRP = 4                   # windows per ACT op / psum tile
MARGIN = 0.4
SCALE = 20.0
EPS = 1e-07
SA = 32.0                 # fp8 pre-scale for a_hat
SW = 16.0                 # fp8 pre-scale for w_hat
ALPHA = SCALE / (SA * SW) # ACT scale

CS = C // NCORES                      # 12500
CSP = ((CS + NW - 1) // NW) * NW      # 12800
NWIN = CSP // NW                      # 25

_CACHE: dict = {}


def _groups(nwin):
    gs, t = [], 0
    while t < nwin:
        g = min(GRP, nwin - t)
        gs.append((t, g))
        t += g
    return gs


def build_kernel(csp, reps=1):
    """reps>1: timing variant — full kernel body repeated inside one program."""
    import concourse.mybir as mybir
    import concourse.tile as tile
    from concourse import bacc

    dt = mybir.dt
    nwin = csp // NW
    nbt = B // 128
    groups = _groups(nwin)
    nsw = len(groups)
    WIN_B = (SUB // 2) * 2 * NW       # 3072 fp8 bytes per window per partition

    nc = bacc.Bacc(None, target_bir_lowering=False)
    at_ext = nc.declare_dram_parameter("at", [128, SUB * B], dt.float8e4, isOutput=False)
    w8_ext = nc.declare_dram_parameter("w8", [128, nwin * WIN_B], dt.float8e4, isOutput=False)
    out_ext = nc.declare_dram_parameter("out", [128, nsw * nbt], dt.float32, isOutput=True)

    with tile.TileContext(nc) as tc:
        with (
            tc.tile_pool(name="atp", bufs=2) as at_pool,
            tc.tile_pool(name="zp", bufs=2) as z_pool,
            tc.tile_pool(name="wload", bufs=3) as w_pool,
            tc.tile_pool(name="scr", bufs=2) as sc_pool,
            tc.tile_pool(name="ps", bufs=2, space="PSUM") as ps_pool,
        ):
            for _ in range(reps):
                at = at_pool.tile([128, SUB, B], dt.float8e4, tag="at")
                nc.scalar.dma_start(out=at[:, :, :], in_=at_ext[:, :])
                zacc = z_pool.tile([128, nsw * nbt], dt.float32, tag="zacc")

                for s, (t0, g) in enumerate(groups):
                    wt = w_pool.tile([128, GRP, SUB // 2, 2, NW], dt.float8e4, tag="wt")
                    nc.sync.dma_start(
                        out=wt[:, :g, :, :, :],
                        in_=w8_ext[:, t0 * WIN_B:(t0 + g) * WIN_B],
                    )
                    for bt in range(nbt):
                        ps = ps_pool.tile([128, GRP * NW], dt.float32, tag="ps")
                        for j in range(SUB // 2):
                            for q in range(g):
                                nc.tensor.matmul(
                                    ps[:, q * NW:(q + 1) * NW],
                                    at[:, 2 * j:2 * j + 2, bt * 128:(bt + 1) * 128],
                                    wt[:, q, j, :, :],
                                    start=(j == 0), stop=(j == SUB // 2 - 1),
                                    perf_mode=mybir.MatmulPerfMode.DoubleRow,
                                )
                        sc = sc_pool.tile([128, GRP * NW], dt.bfloat16, tag="sc")
                        nc.scalar.activation(
                            sc[:, :g * NW], ps[:, :g * NW],
                            mybir.ActivationFunctionType.Exp,
                            scale=ALPHA,
                            accum_out=zacc[:, s * nbt + bt:s * nbt + bt + 1],
                        )

                nc.sync.dma_start(out=out_ext[:, :], in_=zacc[:])

    return nc


def _get_graph(csp, reps=1):
    key = (csp, reps)
    if key not in _CACHE:
        nc = build_kernel(csp, reps)
        nc.finalize()
        _CACHE[key] = nc
    return _CACHE[key]


def _prep_at(embeddings):
    emb = np.asarray(embeddings, dtype=np.float32)
    an = emb / np.linalg.norm(emb, axis=1, keepdims=True)
    at8 = (SA * an).astype(ml_dtypes.float8_e4m3)       # [B, D]
    atT = np.ascontiguousarray(at8.T)                   # [D, B]
    at_r = atT.reshape(SUB, 128, B).transpose(1, 0, 2).reshape(128, SUB * B)
    return np.ascontiguousarray(at_r), an, at8


def _prep_w(W, csp):
    """fp8 shards laid out [p, t, jj, r, n]: value at class t*NW+n,
    k = (2*jj+r)*128+p."""
    Wf = np.asarray(W, dtype=np.float32)
    n = np.linalg.norm(Wf, axis=1, keepdims=True)
    Wn = Wf / n
    w8 = (SW * Wn).astype(ml_dtypes.float8_e4m3)        # [C, D]
    nwin = csp // NW
    shards = []
    for c in range(NCORES):
        sh = np.zeros((csp, D), dtype=ml_dtypes.float8_e4m3)  # pad rows -> 0
        sh[:CS] = w8[c * CS:(c + 1) * CS]
        cT = np.ascontiguousarray(sh.T)                 # [D, csp]
        c5 = cT.reshape(SUB // 2, 2, 128, nwin, NW)     # [jj, r, p, t, n]
        pr = c5.transpose(2, 3, 0, 1, 4).reshape(128, nwin * (SUB // 2) * 2 * NW)
        shards.append(np.ascontiguousarray(pr))
    return shards, w8


def make_in_maps(embeddings, W, csp):
    at_r, an, at8 = _prep_at(embeddings)
    shards, w8 = _prep_w(W, csp)
    in_maps = [{"at": at_r, "w8": shards[c]} for c in range(NCORES)]
    return in_maps, (an, at8, w8)


def finalize(results, aux, W, labels, csp):
    an, at8, w8 = aux
    Wf = np.asarray(W, dtype=np.float32)
    labels = np.asarray(labels).astype(np.int64)
    nwin = csp // NW
    nsw = len(_groups(nwin))
    nbt = B // 128
    Z = np.zeros(B, dtype=np.float64)
    for r in results:
        o = r["out"].astype(np.float64).reshape(128, nsw, nbt).sum(axis=1)
        Z += o.T.reshape(B)
    # padding rows are all-zero fp8 -> each contributes exp(0) = 1
    Z -= float(NCORES * (csp - CS))

    # label-class corrections: remove the device's quantized label term,
    # add the exact margin term. Device label term = exp(ALPHA * a8 . w8_l).
    a8f = at8.astype(np.float64)
    w8l = w8[labels].astype(np.float64)
    cos_q = np.sum(a8f * w8l, axis=1)                   # = SA*SW*cos_quant
    dev_label = np.exp(ALPHA * cos_q)

    wl = Wf[labels]
    wln = wl / np.linalg.norm(wl, axis=1, keepdims=True)
    cos_l = np.sum(an.astype(np.float64) * wln.astype(np.float64), axis=1)
    cos_l = np.clip(cos_l, -1.0 + EPS, 1.0 - EPS)
    t = np.cos(np.arccos(cos_l) + MARGIN) * SCALE
    Z = Z - dev_label + np.exp(t)
    loss = np.mean(np.log(Z) - t)
    return np.asarray(loss, dtype=np.float32)


def kernel(embeddings, labels, W):
    from concourse.bass_utils import run_bass_kernel_spmd

    nc = _get_graph(CSP)
    in_maps, aux = make_in_maps(embeddings, W, CSP)
    res = run_bass_kernel_spmd(nc, in_maps, core_ids=list(range(NCORES)))
    return finalize(res.results, aux, W, labels, CSP)
